# revision 1
# baseline (speedup 1.0000x reference)
"""Causal-mask multi-head attention (B=2, S=2048, D=1024, H=16) on 8 trn2
NeuronCores.

Sharding: core c = 4*b + g handles batch b and head-group g (4 heads).
Each core computes q/k/v projections for its head group (column-sliced
weights), block-causal attention over its batch, and the partial output
projection with its row-slice of wo.  The host sums the 4 per-batch
partials (tensor-parallel partial-sum gather) -- no device collectives.

Device kernel layout notes:
  - All matmul operands are float32r (FP22 multiply, fp32 accumulate).
  - Projections consume host-transposed xT [1024, 2048] so the contraction
    dim (d_model) is on partitions for both operands.
  - Attention computes transposed logits ST = [keys, q] so that P^T is
    directly usable as the moving operand of the AV matmul.
  - The softmax denominator comes for free from a 65th "ones" column on v
    (output row 64 of the AV psum tile).  No max-subtraction: logits are
    ~N(0,1) after the 1/8 scale, far from fp32 exp overflow.
  - Diagonal (partially masked) key blocks are zeroed post-exp with a 0/1
    mask multiplied on the vector engine.
"""
import numpy as np
import ml_dtypes

_bf16 = ml_dtypes.bfloat16

import concourse.bass as bass
import concourse.tile as tile
import concourse.mybir as mybir
from concourse import bacc
from concourse.bass_utils import run_bass_kernel_spmd

B, S, D = 2, 2048, 1024
H, DH = 16, 64
HG = 4                 # heads per core
DG = HG * DH           # 256 projection cols per core
P = 128
QW = 512               # query window (matmul N)
NQW = S // QW          # 4
NKB = S // P           # 16 key blocks
NC = D // P            # 8 contraction chunks of d_model
NRC = S // P           # 16 row chunks
F32 = mybir.dt.float32
F32R = mybir.dt.float32r

_cached_nc = None


def _build_nc():
    nc = bacc.Bacc("TRN2", target_bir_lowering=False, debug=False, num_devices=8)

    BF16 = mybir.dt.bfloat16
    xqT = nc.dram_tensor("xqT", [D, S], BF16, kind="ExternalInput").ap()
    xkT = nc.dram_tensor("xkT", [D, S], BF16, kind="ExternalInput").ap()
    xvT = nc.dram_tensor("xvT", [D, S], BF16, kind="ExternalInput").ap()
    wq = nc.dram_tensor("wq", [D, DG], mybir.dt.bfloat16, kind="ExternalInput").ap()
    wk = nc.dram_tensor("wk", [D, DG], mybir.dt.bfloat16, kind="ExternalInput").ap()
    wv = nc.dram_tensor("wv", [D, DG], mybir.dt.bfloat16, kind="ExternalInput").ap()
    bq = nc.dram_tensor("bq", [DG], F32, kind="ExternalInput").ap()
    bk = nc.dram_tensor("bk", [DG], F32, kind="ExternalInput").ap()
    bv = nc.dram_tensor("bv", [DG], F32R, kind="ExternalInput").ap()
    wo = nc.dram_tensor("wo", [DG, D], F32R, kind="ExternalInput").ap()
    bo = nc.dram_tensor("bo", [D], F32, kind="ExternalInput").ap()
    maskb = nc.dram_tensor("maskb", [P, P], F32R, kind="ExternalInput").ap()
    ident = nc.dram_tensor("ident", [P, P], F32R, kind="ExternalInput").ap()
    onesd = nc.dram_tensor("onesd", [1, P], F32R, kind="ExternalInput").ap()
    out = nc.dram_tensor("out", [S, D], F32, kind="ExternalOutput").ap()

    from contextlib import ExitStack
    with tile.TileContext(nc) as tc, ExitStack() as ctx:
        consts = ctx.enter_context(tc.tile_pool(name="consts", bufs=1))
        slabs = ctx.enter_context(tc.tile_pool(name="slabs", bufs=10))
        persist = ctx.enter_context(tc.tile_pool(name="persist", bufs=1))

        # ---- constants / weights in SBUF (issue order = need order) ----
        wv_sb = consts.tile([P, NC, DG], mybir.dt.bfloat16)
        wv_r = wv.rearrange("(c p) m -> p c m", p=P)
        nc.sync.dma_start(wv_sb[:, 0:1, :], wv_r[:, 0:1, :])
        maskb_sb = consts.tile([P, P], F32R)
        nc.sync.dma_start(maskb_sb[:], maskb)
        ident_sb = consts.tile([P, P], F32R)
        nc.sync.dma_start(ident_sb[:], ident)
        bv_sb = consts.tile([1, DG], F32R)
        ones1 = consts.tile([1, P], F32R)

        # persistent activation storage
        qT = [persist.tile([P, S], F32R, tag=f"qT{t}", name=f"qT{t}") for t in range(2)]
        kT = [persist.tile([P, S], F32R, tag=f"kT{t}", name=f"kT{t}") for t in range(2)]
        v4 = persist.tile([P, NRC, HG, DH + 1], F32R, tag="v4")
        aoT = [persist.tile([P, S], F32R, tag=f"aoT{t}", name=f"aoT{t}") for t in range(2)]
        nc.vector.tensor_scalar(
            out=v4[:, :, :, DH:DH + 1].rearrange("p a b c -> p (a b c)"),
            in0=maskb_sb[:, 0:NRC * HG],
            scalar1=0.0, scalar2=1.0,
            op0=mybir.AluOpType.mult, op1=mybir.AluOpType.add)

        # ---- phase 1: projections (v first, so the q/k psum pool's last
        # users are the q/k evacs and attention isn't gated on v) ----
        with tc.tile_pool(name="ps1", bufs=1, space="PSUM") as ps1:
            # v: rows = seq (16 chunks), cols = head dims (256).
            # One PSUM bank per rowchunk; two passes of 8 rowchunks.
            vslabs = []
            for c in range(NC):
                slab = slabs.tile([P, S], mybir.dt.bfloat16, tag="slab")
                nc.sync.dma_start(slab[:], xvT[c * P:(c + 1) * P, :])
                vslabs.append(slab)
                if c == 0:
                    nc.sync.dma_start(wv_sb[:, 1:NC, :], wv_r[:, 1:NC, :])
                if c == 1:
                    nc.sync.dma_start(bv_sb[:], bv[None, :])
                    nc.sync.dma_start(ones1[:], onesd)
            for vpass in range(2):
                ps = [ps1.tile([P, DG], F32, tag=f"ps1_{i}", name=f"ps1_{i}")
                      for i in range(8)]
                for c in range(NC):
                    for i in range(8):
                        rc = vpass * 8 + i
                        nc.tensor.matmul(
                            ps[i][:],
                            vslabs[c][:, rc * P:(rc + 1) * P],
                            wv_sb[:, c, :],
                            start=(c == 0), stop=False,
                        )
                for i in range(8):
                    rc = vpass * 8 + i
                    nc.tensor.matmul(
                        ps[i][:],
                        ones1[:, :],
                        bv_sb[:, :],
                        start=False, stop=True, skip_group_check=True,
                    )
                    if i % 2 == 0:
                        nc.vector.tensor_copy(
                            out=v4[:, rc, :, 0:DH],
                            in_=ps[i][:].rearrange("p (h d) -> p h d", h=HG),
                        )
                    else:
                        nc.scalar.copy(
                            out=v4[:, rc, :, 0:DH],
                            in_=ps[i][:].rearrange("p (h d) -> p h d", h=HG),
                        )
            # qT then kT: out rows = head dims (2 tiles of 128), cols = seq
            wq_sb = consts.tile([P, NC, DG], mybir.dt.bfloat16)
            nc.sync.dma_start(wq_sb[:], wq.rearrange("(c p) m -> p c m", p=P))
            bq_sb = consts.tile([P, 2], F32)
            nc.sync.dma_start(bq_sb[:], bq.rearrange("(t p) -> p t", p=P))
            wk_sb = consts.tile([P, NC, DG], mybir.dt.bfloat16)
            nc.sync.dma_start(wk_sb[:], wk.rearrange("(c p) m -> p c m", p=P))
            bk_sb = consts.tile([P, 2], F32)
            nc.sync.dma_start(bk_sb[:], bk.rearrange("(t p) -> p t", p=P))
            for name, src, w_sb, b_sb, dst in (
                ("q", xqT, wq_sb, bq_sb, qT),
                ("k", xkT, wk_sb, bk_sb, kT),
            ):
                ps = [ps1.tile([P, QW], F32, tag=f"ps1_{i}", name=f"ps1_{i}") for i in range(8)]
                for c in range(NC):
                    slab = slabs.tile([P, S], mybir.dt.bfloat16, tag="slab")
                    nc.sync.dma_start(slab[:, 0:S // 2],
                                      src[c * P:(c + 1) * P, 0:S // 2])
                    nc.sync.dma_start(slab[:, S // 2:S],
                                      src[c * P:(c + 1) * P, S // 2:S])
                    for t in range(2):
                        for w in range(NQW):
                            nc.tensor.matmul(
                                ps[t * NQW + w][:],
                                w_sb[:, c, t * P:(t + 1) * P],
                                slab[:, w * QW:(w + 1) * QW],
                                start=(c == 0), stop=(c == NC - 1),
                            )
                for t in range(2):
                    for w in range(NQW):
                        if w % 2 == 0:
                            nc.vector.tensor_scalar_add(
                                dst[t][:, w * QW:(w + 1) * QW],
                                ps[t * NQW + w][:],
                                b_sb[:, t:t + 1],
                            )
                        else:
                            nc.scalar.activation(
                                dst[t][:, w * QW:(w + 1) * QW],
                                ps[t * NQW + w][:],
                                mybir.ActivationFunctionType.Identity,
                                bias=b_sb[:, t:t + 1],
                            )

            wo_sb = consts.tile([P, 2, D], F32R)
            nc.sync.dma_start(wo_sb[:], wo.rearrange("(c p) m -> p c m", p=P))
            bo_bc = consts.tile([P, D], F32)
            nc.sync.dma_start(bo_bc[:], bass.AP(
                tensor=bo.tensor, offset=0, ap=[[0, P], [1, D]]))

        # ---- phase 2+3: attention with interleaved output projection ----
        with tc.tile_pool(name="st_ps", bufs=2, space="PSUM") as st_ps, \
             tc.tile_pool(name="ot_ps", bufs=3, space="PSUM") as ot_ps, \
             tc.tile_pool(name="po", bufs=1, space="PSUM") as po, \
             tc.tile_pool(name="ptp", bufs=5) as ptp, \
             tc.tile_pool(name="smp", bufs=4) as smp, \
             tc.tile_pool(name="osb", bufs=4) as osb:
            for qm in range(NQW):
                nkb = 4 * qm + 4
                for hp in range(2):      # head pair = partition halves
                    ot = [ot_ps.tile([DH + 1, QW], F32, tag="ot", name=f"ot{hh}")
                          for hh in range(2)]
                    for kb in range(nkb):
                        st = st_ps.tile([P, 2 * QW], F32, tag="st")
                        joff = kb - 4 * qm
                        # columns below v0 are fully masked for this key
                        # block and never read downstream: skip them in
                        # ST, exp and AV entirely.
                        v0 = max(joff, 0) * P
                        for hh in range(2):
                            lo, hi = hh * DH, (hh + 1) * DH
                            nc.tensor.matmul(
                                st[:, hh * QW + v0:(hh + 1) * QW],
                                kT[hp][lo:hi, kb * P:(kb + 1) * P],
                                qT[hp][lo:hi, qm * QW + v0:(qm + 1) * QW],
                                start=True, stop=(joff < 0),
                            )
                            if joff >= 0:
                                # additive -1e9 triangle on the diagonal
                                # 128-col strip via PE accumulation
                                nc.tensor.matmul(
                                    st[:, hh * QW + v0:hh * QW + v0 + P],
                                    ident_sb[:],
                                    maskb_sb[:],
                                    start=False, stop=True,
                                    skip_group_check=True,
                                )
                        pt = ptp.tile([P, 2 * QW], F32R, tag="pt")
                        if v0 == 0:
                            nc.scalar.activation(
                                pt[:], st[:],
                                mybir.ActivationFunctionType.Exp, scale=0.125)
                        else:
                            for hh in range(2):
                                nc.scalar.activation(
                                    pt[:, hh * QW + v0:(hh + 1) * QW],
                                    st[:, hh * QW + v0:(hh + 1) * QW],
                                    mybir.ActivationFunctionType.Exp,
                                    scale=0.125)
                        for hh in range(2):
                            nc.tensor.matmul(
                                ot[hh][:, v0:QW],
                                v4[:, kb, hp * 2 + hh, :],
                                pt[:, hh * QW + v0:(hh + 1) * QW],
                                start=(kb == 0), stop=(kb == nkb - 1),
                            )
                    for hh in range(2):
                        rcp = smp.tile([1, QW], F32, tag="rcp")
                        nc.vector.reciprocal(rcp[:], ot[hh][DH:DH + 1, :])
                        bc = smp.tile([DH, QW], F32, tag="bc")
                        nc.gpsimd.partition_broadcast(bc[:], rcp[:])
                        nc.vector.tensor_tensor(
                            out=aoT[hp][hh * DH:(hh + 1) * DH,
                                        qm * QW:(qm + 1) * QW],
                            in0=ot[hh][0:DH, :],
                            in1=bc[:],
                            op=mybir.AluOpType.mult,
                        )
                # output projection for this qm's 4 rowchunks
                for rc in range(4 * qm, 4 * qm + 4):
                    o_sb = osb.tile([P, D], F32, tag="o_sb")
                    for nn in range(2):
                        pso = po.tile([P, QW], F32, tag="pso")
                        for hp in range(2):
                            nc.tensor.matmul(
                                pso[:],
                                aoT[hp][:, rc * P:(rc + 1) * P],
                                wo_sb[:, hp, nn * QW:(nn + 1) * QW],
                                start=(hp == 0), stop=(hp == 1),
                            )
                        nc.vector.tensor_tensor(
                            out=o_sb[:, nn * QW:(nn + 1) * QW],
                            in0=pso[:],
                            in1=bo_bc[:, nn * QW:(nn + 1) * QW],
                            op=mybir.AluOpType.add,
                        )
                    nc.sync.dma_start(out[rc * P:(rc + 1) * P, :], o_sb[:])

    nc.compile()
    return nc


def _get_nc():
    global _cached_nc
    if _cached_nc is None:
        _cached_nc = _build_nc()
    return _cached_nc


def _shard_inputs(xk, xq, xv, wq, bq, wk, bk, wv, bv, wo, bo):
    f32 = np.float32
    maskb = np.zeros((P, P), f32)
    for k in range(P):
        maskb[k, :k] = -1.0e9
    ident = np.eye(P, dtype=f32)
    in_maps = []
    for c in range(8):
        b, g = divmod(c, 4)
        gs = slice(g * DG, (g + 1) * DG)
        in_maps.append({
            "xqT": np.ascontiguousarray(np.asarray(xq[b], f32).T.astype(_bf16)),
            "xkT": np.ascontiguousarray(np.asarray(xk[b], f32).T.astype(_bf16)),
            "xvT": np.ascontiguousarray(np.asarray(xv[b], f32).T.astype(_bf16)),
            "wq": np.ascontiguousarray(np.asarray(wq[:, gs], f32).astype(_bf16)),
            "wk": np.ascontiguousarray(np.asarray(wk[:, gs], f32).astype(_bf16)),
            "wv": np.ascontiguousarray(np.asarray(wv[:, gs], f32).astype(_bf16)),
            "bq": np.ascontiguousarray(np.asarray(bq[gs], f32)),
            "bk": np.ascontiguousarray(np.asarray(bk[gs], f32)),
            "bv": np.ascontiguousarray(np.asarray(bv[gs], f32)),
            "wo": np.ascontiguousarray(np.asarray(wo[gs, :], f32)),
            "bo": np.asarray(bo, f32) if g == 0 else np.zeros(D, f32),
            "maskb": maskb,
            "ident": ident,
            "onesd": np.ones((1, P), f32),
        })
    return in_maps


def kernel(xk, xq, xv, wq, bq, wk, bk, wv, bv, wo, bo, _trace=False):
    nc = _get_nc()
    in_maps = _shard_inputs(xk, xq, xv, wq, bq, wk, bk, wv, bv, wo, bo)
    res = run_bass_kernel_spmd(nc, in_maps, core_ids=list(range(8)),
                               trace=_trace)
    parts = [r["out"] for r in res.results]
    out = np.stack([
        parts[0] + parts[1] + parts[2] + parts[3],
        parts[4] + parts[5] + parts[6] + parts[7],
    ]).astype(np.float32)
    if _trace:
        kernel._last_results = res
    return out



# revision 9
# speedup vs baseline: 1.1884x; 1.1884x over previous
"""Causal-mask multi-head attention (B=2, S=2048, D=1024, H=16) on 8 trn2
NeuronCores.

Sharding: core c = 4*b + g handles batch b and head-group g (4 heads).
No device collectives; the host sums the 4 per-batch partial outputs.

Perf design (vs the f32r baseline):
  - All heavy matmuls run as fp8e4m3 DoubleRow (2 k-tiles per instruction,
    0.5 PE cycles per output column = 4x bf16 throughput).
  - Accuracy is restored by host-side hi/lo error compensation:
      * inputs are pre-scaled by 32 so e4m3 residuals stay in normal range
      * q/k projections: terms (xh,wh)+(xh,wl)  [x-residual skipped: the
        softmax rows that would notice sit in the first query window]
      * the first query window (qm=0) instead uses a fully compensated
        3-term bf16 path (separate qTb/kTb tiles) for ST/exp/AV
      * v projection: (xh,wvh)+(xl,wvh)+(xh/64,64*wvl); v stored as
        fp8 hi+lo pair, consumed by a single DoubleRow AV matmul
  - Softmax: exp(logit-4) so probabilities fit e4m3 range; the constant
    cancels via the ones-row denominator.  Head-dim-32 DoubleRow ST uses a
    host-side column permutation of wq/wk so the projection psum evacuates
    straight into the [32-partition, 2-ktile] layout with same-lane copies.
  - The PE instruction stream is hand-woven (projections / ST / AV /
    output projection interleaved) so the PE never idles behind the
    Activation engine, which is the ~55us exp roofline for this shape.
"""
import numpy as np
import ml_dtypes

import concourse.bass as bass
import concourse.tile as tile
import concourse.mybir as mybir
from concourse import bacc
from concourse.bass_utils import run_bass_kernel_spmd

_bf16 = ml_dtypes.bfloat16
_e4m3 = ml_dtypes.float8_e4m3

B, S, D = 2, 2048, 1024
H, DH = 16, 64
HG = 4                 # heads per core
DG = HG * DH           # 256 projection cols per core
P = 128
QW = 512               # query window
NQW = S // QW          # 4
NC = D // P            # 8 contraction chunks of d_model
NRC = S // P           # 16 seq row chunks
F32 = mybir.dt.float32
F32R = mybir.dt.float32r
BF16 = mybir.dt.bfloat16
FP8 = mybir.dt.float8e4
DR = mybir.MatmulPerfMode.DoubleRow

EXPS = 0.125 / 1024.0  # logits are (32q).(32k); true logit = raw * EXPS
EXPB = -4.0            # keeps exp() within e4m3 range; cancels in softmax

_cached_nc = None


def _rep2(ap):
    """[P, N] AP -> [P, 2, N] with a stride-0 middle dim (same data in both
    DoubleRow slots)."""
    return bass.AP(tensor=ap.tensor, offset=ap.offset,
                   ap=[ap.ap[0], [0, 2]] + list(ap.ap[1:]))


def _build_nc():
    nc = bacc.Bacc("TRN2", target_bir_lowering=False, debug=False, num_devices=8)

    # moving-side activations, contraction-major [D, S]
    xkh = nc.dram_tensor("xkh", [D, S], FP8, kind="ExternalInput").ap()
    xqh = nc.dram_tensor("xqh", [D, S], FP8, kind="ExternalInput").ap()
    xkl0 = nc.dram_tensor("xkl0", [D, QW], FP8, kind="ExternalInput").ap()
    xql0 = nc.dram_tensor("xql0", [D, QW], FP8, kind="ExternalInput").ap()
    xvh = nc.dram_tensor("xvh", [D, S], FP8, kind="ExternalInput").ap()
    xvl = nc.dram_tensor("xvl", [D, S], FP8, kind="ExternalInput").ap()
    xvh64 = nc.dram_tensor("xvh64", [D, S], FP8, kind="ExternalInput").ap()
    # weights, host-packed p-major [P, NC*DG]
    wkh = nc.dram_tensor("wkh", [P, NC * DG], FP8, kind="ExternalInput").ap()
    wkl = nc.dram_tensor("wkl", [P, NC * DG], FP8, kind="ExternalInput").ap()
    wqh = nc.dram_tensor("wqh", [P, NC * DG], FP8, kind="ExternalInput").ap()
    wql = nc.dram_tensor("wql", [P, NC * DG], FP8, kind="ExternalInput").ap()
    wvh = nc.dram_tensor("wvh", [P, NC * DG], FP8, kind="ExternalInput").ap()
    wvl64 = nc.dram_tensor("wvl64", [P, NC * DG], FP8, kind="ExternalInput").ap()
    bqp = nc.dram_tensor("bqp", [DG], F32, kind="ExternalInput").ap()
    bkp = nc.dram_tensor("bkp", [DG], F32, kind="ExternalInput").ap()
    bvr = nc.dram_tensor("bvr", [DG], F32R, kind="ExternalInput").ap()
    onesd = nc.dram_tensor("onesd", [1, P], F32R, kind="ExternalInput").ap()
    maskb = nc.dram_tensor("maskb", [P, P], BF16, kind="ExternalInput").ap()
    ident = nc.dram_tensor("ident", [P, P], BF16, kind="ExternalInput").ap()
    wob = nc.dram_tensor("wob", [P, 2 * D], BF16, kind="ExternalInput").ap()
    out = nc.dram_tensor("out", [S, D], BF16, kind="ExternalOutput").ap()

    from contextlib import ExitStack
    with tile.TileContext(nc) as tc, ExitStack() as ctx:
        consts = ctx.enter_context(tc.tile_pool(name="consts", bufs=1))
        slabs = ctx.enter_context(tc.tile_pool(name="slabs", bufs=1))
        persist = ctx.enter_context(tc.tile_pool(name="persist", bufs=1))
        work = ctx.enter_context(tc.tile_pool(name="work", bufs=1))
        ps = ctx.enter_context(tc.tile_pool(name="ps", bufs=1, space="PSUM"))

        # ---- consts (issue order = DMA arrival order = need order) ----
        wkh_sb = consts.tile([P, NC, DG], FP8)
        nc.sync.dma_start(wkh_sb[:].rearrange("p c m -> p (c m)"), wkh)
        wkl_sb = consts.tile([P, NC, DG], FP8)
        nc.sync.dma_start(wkl_sb[:].rearrange("p c m -> p (c m)"), wkl)
        wqh_sb = consts.tile([P, NC, DG], FP8)
        nc.sync.dma_start(wqh_sb[:].rearrange("p c m -> p (c m)"), wqh)
        wql_sb = consts.tile([P, NC, DG], FP8)
        nc.sync.dma_start(wql_sb[:].rearrange("p c m -> p (c m)"), wql)
        bk_sb = consts.tile([P, 2], F32)
        nc.sync.dma_start(bk_sb[:], bkp.rearrange("(t p) -> p t", p=P))
        bq_sb = consts.tile([P, 2], F32)
        nc.sync.dma_start(bq_sb[:], bqp.rearrange("(t p) -> p t", p=P))
        maskb_sb = consts.tile([P, P], BF16)
        nc.sync.dma_start(maskb_sb[:], maskb)
        ident_sb = consts.tile([P, P], BF16)
        nc.sync.dma_start(ident_sb[:], ident)
        ones1 = consts.tile([1, P], F32R)
        nc.sync.dma_start(ones1[:], onesd)
        bv_sb = consts.tile([1, DG], F32R)
        nc.sync.dma_start(bv_sb[:], bvr[None, :])

        # ---- activation slabs, in arrival order ----
        xkh_sb = slabs.tile([P, NC, S], FP8, tag="slab", bufs=4, name="xkh")
        for c in range(NC):
            nc.sync.dma_start(xkh_sb[:, c, :], xkh[c * P:(c + 1) * P, :])
        xqh_sb = slabs.tile([P, NC, S], FP8, tag="slab", bufs=4, name="xqh")
        for c in range(NC):
            nc.sync.dma_start(xqh_sb[:, c, :], xqh[c * P:(c + 1) * P, :])
        xkl0_sb = slabs.tile([P, NC, QW], FP8, tag="xkl0")
        for c in range(NC):
            nc.sync.dma_start(xkl0_sb[:, c, :], xkl0[c * P:(c + 1) * P, :])
        xql0_sb = slabs.tile([P, NC, QW], FP8, tag="xql0")
        for c in range(NC):
            nc.sync.dma_start(xql0_sb[:, c, :], xql0[c * P:(c + 1) * P, :])
        wvh_sb = consts.tile([P, NC, DG], FP8)
        nc.sync.dma_start(wvh_sb[:].rearrange("p c m -> p (c m)"), wvh)
        wvl64_sb = consts.tile([P, NC, DG], FP8)
        nc.sync.dma_start(wvl64_sb[:].rearrange("p c m -> p (c m)"), wvl64)
        xvh_sb = slabs.tile([P, NC, S], FP8, tag="slab", bufs=4, name="xvh")
        for c in range(NC):
            nc.sync.dma_start(xvh_sb[:, c, :], xvh[c * P:(c + 1) * P, :])
        xvl_sb = slabs.tile([P, NC, S], FP8, tag="slab", bufs=4, name="xvl")
        for c in range(NC):
            nc.sync.dma_start(xvl_sb[:, c, :], xvl[c * P:(c + 1) * P, :])
        xvh64_sb = slabs.tile([P, NC, S], FP8, tag="slab", bufs=4, name="xvh64")
        for c in range(NC):
            nc.sync.dma_start(xvh64_sb[:, c, :], xvh64[c * P:(c + 1) * P, :])
        wo_sb = consts.tile([P, 2, D], BF16)
        nc.sync.dma_start(wo_sb[:].rearrange("p c m -> p (c m)"), wob)

        # ---- persistent activations ----
        # partition 32*(g%2)+p <-> (head g, dim t*32+p) at ktile t; heads
        # 0,1 in tile [0], heads 2,3 in tile [1] (base 96 is not a legal
        # matmul operand base, so four 32-row heads cannot share one tile)
        qT8 = [persist.tile([P, 2, S], FP8, tag=f"qT8{i}", name=f"qT8{i}")
               for i in range(2)]
        kT8 = [persist.tile([P, 2, S], FP8, tag=f"kT8{i}", name=f"kT8{i}")
               for i in range(2)]
        qTb = [persist.tile([P, 2, QW], BF16, tag=f"qTb{i}", name=f"qTb{i}")
               for i in range(2)]
        kTb = [persist.tile([P, 2, QW], BF16, tag=f"kTb{i}", name=f"kTb{i}")
               for i in range(2)]
        # v4: [seq-part, rowchunk, head, hi/lo, dh+ones(+pad)]
        # last dim padded 65->80: DoubleRow lhsT outer free stride must be
        # even and 16B-aligned (s3_lw_dual_fp8_restrictions)
        v4 = persist.tile([P, NRC, HG, 2, 80], FP8, tag="v4")
        aoT = [persist.tile([P, S], BF16, tag=f"aoT{t}", name=f"aoT{t}")
               for t in range(2)]
        nc.vector.memset(v4[:, :, :, 0, DH:DH + 1], 1.0)
        nc.vector.memset(v4[:, :, :, 1, DH:DH + 1], 0.0)
        expb = consts.tile([P, 1], F32)
        nc.vector.memset(expb[:], EXPB)

        mm = nc.tensor.matmul

        # ---------------- helpers ----------------
        def qk_hi_tile(w_hi, w_lo, x_h, dst8, b_sb, t, w):
            pp = ps.tile([P, QW], F32, tag="pp", bufs=2, name=f"pp_{t}_{w}")
            ts, te = t * P, (t + 1) * P
            for term, wsb in ((0, w_hi), (1, w_lo)):
                for c4 in range(4):
                    mm(pp[:],
                       wsb[:, 2 * c4:2 * c4 + 2, ts:te],
                       x_h[:, 2 * c4:2 * c4 + 2, w * QW:(w + 1) * QW],
                       start=(term == 0 and c4 == 0),
                       stop=(term == 1 and c4 == 3),
                       perf_mode=DR)
            for i in range(2):
                nc.vector.tensor_scalar(
                    out=dst8[i][0:64, t, w * QW:(w + 1) * QW],
                    in0=pp[64 * i:64 * i + 64, :],
                    scalar1=1.0 / 32.0,
                    scalar2=b_sb[64 * i:64 * i + 64, t:t + 1],
                    op0=mybir.AluOpType.mult, op1=mybir.AluOpType.add)

        def reproj_w0(w_hi, w_lo, x_h, x_l0, dstb, b_sb, t):
            pp = ps.tile([P, QW], F32, tag="pp", bufs=2, name=f"rp_{t}")
            ts, te = t * P, (t + 1) * P
            terms = ((w_hi, x_h), (w_hi, x_l0), (w_lo, x_h))
            for term, (wsb, xsb) in enumerate(terms):
                for c4 in range(4):
                    xs = xsb[:, 2 * c4:2 * c4 + 2, 0:QW]
                    mm(pp[:],
                       wsb[:, 2 * c4:2 * c4 + 2, ts:te], xs,
                       start=(term == 0 and c4 == 0),
                       stop=(term == 2 and c4 == 3),
                       perf_mode=DR)
            for i in range(2):
                nc.vector.tensor_scalar(
                    out=dstb[i][0:64, t, :],
                    in0=pp[64 * i:64 * i + 64, :],
                    scalar1=1.0 / 32.0,
                    scalar2=b_sb[64 * i:64 * i + 64, t:t + 1],
                    op0=mybir.AluOpType.mult, op1=mybir.AluOpType.add)

        def vproj_rc(rc):
            pp = ps.tile([P, DG], F32, tag="pp", bufs=2, name=f"vp_{rc}")
            rs, re = rc * P, (rc + 1) * P
            terms = ((xvh_sb, wvh_sb), (xvl_sb, wvh_sb), (xvh64_sb, wvl64_sb))
            for term, (xsb, wsb) in enumerate(terms):
                for c4 in range(4):
                    mm(pp[:],
                       xsb[:, 2 * c4:2 * c4 + 2, rs:re],
                       wsb[:, 2 * c4:2 * c4 + 2, :],
                       start=(term == 0 and c4 == 0), stop=False,
                       perf_mode=DR)
            mm(pp[:], ones1[:, :], bv_sb[:, :], start=False, stop=True,
               skip_group_check=True)
            hi = v4[:, rc, :, 0, 0:DH]
            nc.vector.tensor_copy(
                out=hi, in_=pp[:].rearrange("p (h d) -> p h d", h=HG))
            nc.vector.tensor_tensor(
                out=v4[:, rc, :, 1, 0:DH],
                in0=pp[:].rearrange("p (h d) -> p h d", h=HG),
                in1=hi, op=mybir.AluOpType.subtract)

        fill = []           # deferred PE/engine work, drained inside ST loops

        def drain(n):
            for _ in range(n):
                if not fill:
                    return
                fill.pop(0)()

        def st_exp(qm, pair, kb, nkb, pts):
            """issue ST (+mask) and exp for one (qm, pair, kb); append pt."""
            joff = kb - 4 * qm
            v0 = max(joff, 0) * P
            st = ps.tile([P, 2 * QW], F32, tag="st", bufs=2,
                         name=f"st{qm}_{pair}_{kb}")
            for j in range(2):
                g = 2 * pair + j
                gi, base = divmod(g, 2)
                base *= 32
                lo = j * QW
                if qm == 0:
                    for t in range(2):
                        mm(st[:, lo + v0:lo + QW],
                           kTb[gi][base:base + 32, t, kb * P:(kb + 1) * P],
                           qTb[gi][base:base + 32, t, v0:QW],
                           start=(t == 0), stop=False)
                else:
                    mm(st[:, lo + v0:lo + QW],
                       kT8[gi][base:base + 32, :, kb * P:(kb + 1) * P],
                       qT8[gi][base:base + 32, :, qm * QW + v0:(qm + 1) * QW],
                       start=True, stop=(joff < 0), perf_mode=DR)
                if joff >= 0:
                    mm(st[:, lo + v0:lo + v0 + P], ident_sb[:], maskb_sb[:],
                       start=False, stop=True, skip_group_check=True)
            if qm == 0:
                pt = work.tile([P, 2 * QW], BF16, tag="ptb", bufs=2,
                               name=f"ptb{pair}_{kb}")
            else:
                pt = work.tile([P, 2 * QW], FP8, tag="pt8", bufs=34,
                               name=f"pt{qm}_{pair}_{kb}")
            if v0 == 0:
                nc.scalar.activation(pt[:], st[:],
                                     mybir.ActivationFunctionType.Exp,
                                     bias=expb[:], scale=EXPS)
            else:
                for j in range(2):
                    lo = j * QW
                    nc.scalar.activation(pt[:, lo + v0:lo + QW],
                                         st[:, lo + v0:lo + QW],
                                         mybir.ActivationFunctionType.Exp,
                                         bias=expb[:], scale=EXPS)
            pts.append((kb, v0, pt))

        def av_one(qm, pair, ot, kb, v0, pt, nkb):
            for j in range(2):
                g = 2 * pair + j
                lo = j * QW
                if qm == 0:
                    for hl in range(2):
                        mm(ot[j][:, v0:QW],
                           v4[:, kb, g, hl, 0:DH + 1],
                           pt[:, lo + v0:lo + QW],
                           start=(kb == 0 and hl == 0),
                           stop=(kb == nkb - 1 and hl == 1))
                else:
                    mm(ot[j][:, v0:QW],
                       v4[:, kb, g, :, 0:DH + 1],
                       _rep2(pt[:, lo + v0:lo + QW]),
                       start=(kb == 0), stop=(kb == nkb - 1),
                       perf_mode=DR)

        def norm_pair(qm, pair, ot):
            for j in range(2):
                rcp = work.tile([1, QW], F32, tag="rcp", bufs=2,
                                name=f"rcp{qm}{pair}{j}")
                nc.vector.reciprocal(rcp[:], ot[j][DH:DH + 1, :])
                bc = work.tile([DH, QW], F32, tag="bc", bufs=2,
                               name=f"bc{qm}{pair}{j}")
                nc.gpsimd.partition_broadcast(bc[:], rcp[:])
                nc.vector.tensor_tensor(
                    out=aoT[pair][j * DH:(j + 1) * DH,
                                  qm * QW:(qm + 1) * QW],
                    in0=ot[j][0:DH, :], in1=bc[:],
                    op=mybir.AluOpType.mult)

        def av_norm_deferred(qm, pair, pts):
            """AV + norm for a fully ST'd (qm, pair); ot from the pp ring."""
            nkb = 4 * qm + 4
            ot = [ps.tile([DH + 1, QW], F32, tag="pp", bufs=2,
                          name=f"otd{qm}{pair}{j}") for j in range(2)]
            for kb, v0, pt in pts:
                av_one(qm, pair, ot, kb, v0, pt, nkb)
            norm_pair(qm, pair, ot)

        def outproj_rc(rc, dve):
            o_sb = work.tile([P, D], BF16, tag="osb", bufs=2, name=f"o{rc}")
            for nn in range(2):
                pp = ps.tile([P, QW], F32, tag="pp", bufs=2, name=f"po{rc}{nn}")
                for pair in range(2):
                    mm(pp[:],
                       aoT[pair][:, rc * P:(rc + 1) * P],
                       wo_sb[:, pair, nn * QW:(nn + 1) * QW],
                       start=(pair == 0), stop=(pair == 1))
                nc.vector.tensor_copy(out=o_sb[:, nn * QW:(nn + 1) * QW],
                                      in_=pp[:])
            nc.sync.dma_start(out[rc * P:(rc + 1) * P, :], o_sb[:])

        # ---------------- PE program ----------------
        # hi-only k/q projections (fp8 path for qm>=1)
        for w in range(NQW):
            for t in range(2):
                qk_hi_tile(wkh_sb, wkl_sb, xkh_sb, kT8, bk_sb, t, w)
        for w in range(NQW):
            for t in range(2):
                qk_hi_tile(wqh_sb, wql_sb, xqh_sb, qT8, bq_sb, t, w)
        # 3-term bf16 re-projection of window 0 (accurate early-row path)
        for t in range(2):
            reproj_w0(wkh_sb, wkl_sb, xkh_sb, xkl0_sb, kTb, bk_sb, t)
        for t in range(2):
            reproj_w0(wqh_sb, wql_sb, xqh_sb, xql0_sb, qTb, bq_sb, t)

        # qm=1: ST only (v4 not ready yet); AV deferred
        pts_q1 = {0: [], 1: []}
        for pair in range(2):
            for kb in range(8):
                st_exp(1, pair, kb, 8, pts_q1[pair])

        # qm=2 pair0: ST, weaving the v projection in from iter 4
        pts_q2 = {0: [], 1: []}
        for rc in range(NRC):
            fill.append(lambda rc=rc: vproj_rc(rc))
        for kb in range(12):
            st_exp(2, 0, kb, 12, pts_q2[0])
            if kb >= 4:
                drain(2)
        # qm=2 pair1: ST, weaving remaining vproj then deferred qm1 AV+norm
        fill.append(lambda: av_norm_deferred(1, 0, pts_q1[0]))
        fill.append(lambda: av_norm_deferred(1, 1, pts_q1[1]))
        for kb in range(12):
            st_exp(2, 1, kb, 12, pts_q2[1])
            drain(1)
        drain(len(fill))

        # qm=0 (bf16 path, inline AV) with qm1 output projection woven in
        for rc in range(4, 8):
            fill.append(lambda rc=rc: outproj_rc(rc, rc % 2 == 0))
        for pair in range(2):
            ot = [ps.tile([DH + 1, QW], F32, tag="ot", bufs=2,
                          name=f"ot0_{pair}{j}") for j in range(2)]
            pts = []
            for kb in range(4):
                st_exp(0, pair, kb, 4, pts)
                kb_, v0_, pt_ = pts[-1]
                av_one(0, pair, ot, kb_, v0_, pt_, 4)
                drain(1)
            norm_pair(0, pair, ot)

        # qm=3 (inline AV), weaving qm2 AV/norm, then qm0/qm2 outproj
        for pair in range(2):
            fill.append(lambda pair=pair: av_norm_deferred(2, pair, pts_q2[pair]))
        for rc in range(0, 4):
            fill.append(lambda rc=rc: outproj_rc(rc, rc % 2 == 0))
        for rc in range(8, 12):
            fill.append(lambda rc=rc: outproj_rc(rc, rc % 2 == 0))
        for pair in range(2):
            ot = [ps.tile([DH + 1, QW], F32, tag="ot", bufs=2,
                          name=f"ot3_{pair}{j}") for j in range(2)]
            pts = []
            for kb in range(16):
                st_exp(3, pair, kb, 16, pts)
                kb_, v0_, pt_ = pts[-1]
                av_one(3, pair, ot, kb_, v0_, pt_, 16)
                drain(1)
            norm_pair(3, pair, ot)
        drain(len(fill))

        # qm=3 output projection tail
        for rc in range(12, 16):
            outproj_rc(rc, rc % 2 == 0)

    nc.compile()
    return nc


def _get_nc():
    global _cached_nc
    if _cached_nc is None:
        _cached_nc = _build_nc()
    return _cached_nc


def _e4(a):
    return np.clip(np.asarray(a, np.float32), -240, 240).astype(_e4m3)


def _pack_w(w):
    """[D, DG] -> p-major [P, NC*DG]"""
    return np.ascontiguousarray(
        w.reshape(NC, P, DG).transpose(1, 0, 2).reshape(P, NC * DG))


def _shard_inputs(xk, xq, xv, wq, bq, wk, bk, wv, bv, wo, bo):
    f32 = np.float32
    # permutation of each head-group's 256 columns: new[t*128+g*32+p] = g*64+t*32+p
    pidx = np.array([g * 64 + t * 32 + p
                     for t in range(2) for g in range(4) for p in range(32)])
    maskb = np.zeros((P, P), f32)
    for k in range(P):
        maskb[k, :k] = -1.0e12
    maskb = maskb.astype(_bf16)
    ident = np.eye(P, dtype=f32).astype(_bf16)
    onesd = np.ones((1, P), f32)

    # per-batch activation prep (shared by the 4 cores of each batch)
    bx = []
    for b in range(B):
        xq32 = np.ascontiguousarray(np.asarray(xq[b], f32).T) * 32.0
        xk32 = np.ascontiguousarray(np.asarray(xk[b], f32).T) * 32.0
        xv32 = np.ascontiguousarray(np.asarray(xv[b], f32).T) * 32.0
        xqh = _e4(xq32)
        xkh = _e4(xk32)
        xvh = _e4(xv32)
        xvl = _e4(xv32 - xvh.astype(f32))
        bx.append({
            "xqh": xqh,
            "xkh": xkh,
            "xql0": _e4((xq32 - xqh.astype(f32))[:, :QW]),
            "xkl0": _e4((xk32 - xkh.astype(f32))[:, :QW]),
            "xvh": xvh,
            "xvl": xvl,
            "xvh64": _e4(xvh.astype(f32) / 64.0),
        })

    in_maps = []
    for c in range(8):
        b, g = divmod(c, 4)
        gs = slice(g * DG, (g + 1) * DG)
        wq32 = np.asarray(wq[:, gs], f32)[:, pidx] * 32.0
        wk32 = np.asarray(wk[:, gs], f32)[:, pidx] * 32.0
        wvg = np.asarray(wv[:, gs], f32)
        wqh = _e4(wq32)
        wkh = _e4(wk32)
        wvh = _e4(wvg)
        m = dict(bx[b])
        m.update({
            "wqh": _pack_w(wqh),
            "wql": _pack_w(_e4(wq32 - wqh.astype(f32))),
            "wkh": _pack_w(wkh),
            "wkl": _pack_w(_e4(wk32 - wkh.astype(f32))),
            "wvh": _pack_w(wvh),
            "wvl64": _pack_w(_e4(64.0 * (wvg - wvh.astype(f32)))),
            "bqp": np.ascontiguousarray(np.asarray(bq[gs], f32)[pidx] * 32.0),
            "bkp": np.ascontiguousarray(np.asarray(bk[gs], f32)[pidx] * 32.0),
            "bvr": np.ascontiguousarray(np.asarray(bv[gs], f32) * 32.0),
            "onesd": onesd,
            "maskb": maskb,
            "ident": ident,
            "wob": np.ascontiguousarray(
                (np.asarray(wo[gs, :], f32) / 32.0)
                .reshape(2, P, D).transpose(1, 0, 2).reshape(P, 2 * D)
                .astype(_bf16)),
        })
        in_maps.append(m)
    return in_maps


def kernel(xk, xq, xv, wq, bq, wk, bk, wv, bv, wo, bo, _trace=False):
    nc = _get_nc()
    in_maps = _shard_inputs(xk, xq, xv, wq, bq, wk, bk, wv, bv, wo, bo)
    res = run_bass_kernel_spmd(nc, in_maps, core_ids=list(range(8)),
                               trace=_trace)
    parts = [np.asarray(r["out"], np.float32) for r in res.results]
    out = np.stack([
        parts[0] + parts[1] + parts[2] + parts[3],
        parts[4] + parts[5] + parts[6] + parts[7],
    ]) + np.asarray(bo, np.float32)[None, None, :]
    if _trace:
        kernel._last_results = res
    return out.astype(np.float32)
